# revision 51
# baseline (speedup 1.0000x reference)
"""Trainium2 Bass kernel for nn_Attention_45037027066352 (sparse_attention).

Reference computation (per batch b, head h; N=1024 tokens, HD=64, H=12):
    qkv   = x @ Wqkv.T                     -> q,k,v [B,H,N,HD]
    Qspk  = relu(q) @ Wfc1.T + bfc1
    Kspk  = relu(k) @ Wfc2.T + bfc2
    att   = softmax(relu(Qspk @ Kspk.T * SCALE) * 2)
    out_h = att @ (relu(v) * 4)
    y     = concat_h(out_h) @ Wproj.T + bproj
Sharding: pure data-parallel over B=8 across the 8 NeuronCores.

Schedule (measured 256.6us vs the 288.9us v1 baseline; rel err 3.5e-3):
  - PE warm-up junk matmuls at t=0 so the DVFS p-state ramps during the
    input DMA; input DMA split across the two HW-DGE queues (x+wqk on SP,
    consts+wv on ACT) so x lands early.
  - Only q,k for pair 0 are produced up front (kc-outer, DMA-paced with
    woven gates); all v tiles and the remaining 10 q/k chunks are WOVEN
    into the attention pair loops as PE filler under the ACT-exp-bound
    windows (v at every jt of pair 0; q/k chunks for pair p+1 at jt 2/5
    of pair p; fc for pair p+1 at jt 6).
  - Per-pair attention is software-pipelined per j-tile: rowsum+PV matmuls
    for tile jt-1 are issued right after the score matmuls for tile jt, so
    the PE never drains waiting for the last exp/max. Scores issue order
    A-h0,B-h0,A-h1,B-h1 -- row-tiled A/B matmuls start back-to-back (pc
    order) and run concurrently. Rowsum A/B and PV A/B pairs are col-tiled
    (A rows 0:64, B rows 64:128).
  - Rowsums keep 64 duplicated rows per head, so the per-pair drain is
    just reciprocal_approx_fast([128,N]) straight off PSUM (the dup rows
    ARE the partition broadcast) + one tensor_tensor normalize -- no DMA
    bounce. Drain for pair p-1 is emitted at pair p's start, behind fc.
  - exp on ACT (the pacing engine, ~2.3us/jt); qs/ks bias-adds on ACT;
    max(.,1) on DVE (GpSimd measured ~16x slower -- do not offload there).
  - Wproj/outT in bf16 (same 1cy/row as f32r, half the LDWEIGHTS cost).
  - fp8 DoubleRow for q/k was tried and REVERTED: the device is power-
    throttle-limited, and the denser matmuls slowed everything else ~25%.

TRN2 Matmult encodes at most ONE sync wait, so every matmul's dependencies
must either be pre-observed by the PE or share one semaphore:
  - each input DMA is "gated" by a tiny PE matmul reading it, and
  - every PSUM tile gets a 1-element DVE memset as its first toucher.
"""

import numpy as np

import concourse.bass as bass
import concourse.bacc as bacc_mod
import concourse.bass_isa as bass_isa
import concourse.mybir as mybir
import concourse.tile as tile
from concourse.bass_utils import run_bass_kernel_spmd

import ml_dtypes

B, N, C, H, HD = 8, 1024, 768, 12, 64
SCALE = HD**-0.5
T_STEPS = 4
N_HALF = T_STEPS // 2

F32 = mybir.dt.float32
F32R = mybir.dt.float32r
BF16 = mybir.dt.bfloat16
FP8 = mybir.dt.float8e4

SX = 8.0  # fp8 pre-scale on x
SW = 32.0  # fp8 pre-scale on Wqkv (q,k rows)

NPAIR = H // 2  # 6 head pairs
KC = C // 128  # 6 contraction chunks for C=768
NT = N // 128  # 8 token tiles
NH = N // 512  # 2 free-dim halves


def build_nc() -> bass.Bass:
    nc = bacc_mod.Bacc()

    xT = nc.dram_tensor("xT", [C, N], BF16, kind="ExternalInput")
    wqkvT = nc.dram_tensor("wqkvT", [C, 3 * C], BF16, kind="ExternalInput")
    wfc1p = nc.dram_tensor("wfc1p", [128, 128], BF16, kind="ExternalInput")
    wfc2p = nc.dram_tensor("wfc2p", [128, 128], BF16, kind="ExternalInput")
    b1p = nc.dram_tensor("b1p", [128, 1], F32, kind="ExternalInput")
    b2p = nc.dram_tensor("b2p", [128, 1], F32, kind="ExternalInput")
    wprojT = nc.dram_tensor("wprojT", [C, C], BF16, kind="ExternalInput")
    bprojp = nc.dram_tensor("bprojp", [128, KC], F32, kind="ExternalInput")

    yT = nc.dram_tensor("yT", [C, N], F32, kind="ExternalOutput")

    xT_v = xT.rearrange("(ko p) n -> p ko n", p=128)
    wqkvT_v = wqkvT.rearrange("(ko p) j -> p ko j", p=128)
    wprojT_v = wprojT.rearrange("(ko p) e -> p ko e", p=128)
    yT_v = yT.rearrange("(eo p) n -> p eo n", p=128)

    with tile.TileContext(nc) as tc:
        with (
            tc.tile_pool(name="consts", bufs=1) as consts,
            tc.tile_pool(name="scps", bufs=2, space="PSUM") as sc_psum,
            tc.tile_pool(name="accps", bufs=2, space="PSUM") as acc_psum,
            tc.tile_pool(name="vr", bufs=1) as vr_pool,
            tc.tile_pool(name="rqk", bufs=1) as rqk_pool,
        ):
            # ---- junk warm-up: keep the PE continuously busy during the
            # initial input DMA so the p-state ramp completes before the
            # first real matmul.
            junk_sb = consts.tile([128, 512], BF16)
            nc.vector.memset(junk_sb[:], 0.0)
            junk_ps = acc_psum.tile([128, 1024], F32, tag="acc", name="junk")
            nc.vector.memset(junk_ps[:, 0:1], 0.0)
            trash_holder = [junk_ps]

            def gate(region, kpart=128):
                # Tiny PE matmul reading a freshly DMA'd SBUF region so the
                # PE observes that DMA queue's semaphore once.
                m = 65 if kpart == 128 else 64
                nc.tensor.matmul(
                    trash_holder[0][0:m, 0:2],
                    lhsT=region[0:kpart, 0:m],
                    rhs=region[0:kpart, 0:2],
                    start=True,
                    stop=True,
                )

            def ps_tile(pool, tag):
                t = pool.tile([128, N], F32, tag=tag)
                nc.vector.memset(t[:, 0:1], 0.0)
                return t

            # ---- constants ----
            wfc1_sb = consts.tile([128, 128], BF16)  # blockdiag(Wfc1.T*2s, ..)
            wfc2_sb = consts.tile([128, 128], BF16)
            b1_sb = consts.tile([128, 1], F32)
            b2_sb = consts.tile([128, 1], F32)
            bproj_sb = consts.tile([128, KC], F32)
            ones_sb = consts.tile([128, HD], BF16)
            nc.vector.memset(ones_sb[:], 1.0)
            # consts + wv ride the Activation HW-DGE queue so the big x/wqk
            # stream on the SP queue isn't delayed behind them
            nc.scalar.dma_start(wfc1_sb[:], wfc1p[:, :])
            nc.scalar.dma_start(wfc2_sb[:], wfc2p[:, :])
            nc.scalar.dma_start(b1_sb[:], b1p[:, :])
            nc.scalar.dma_start(b2_sb[:], b2p[:, :])
            nc.scalar.dma_start(bproj_sb[:], bprojp[:, :])

            # ACT exp table warm-up
            warm_sb = consts.tile([128, 2], F32)
            nc.scalar.activation(
                warm_sb[:], b1_sb[:, 0:1].to_broadcast([128, 2]),
                mybir.ActivationFunctionType.Exp,
            )

            vr_sb = vr_pool.tile([128, NT, C], BF16)  # relu(v)*4, natural
            rqk_sb = rqk_pool.tile([128, 2 * NPAIR, N], BF16)  # relu(qT/kT)

            # ===== phase 1: input DMA + q,k for pair 0; the rest of the
            # qkv projection (all v tiles, remaining q/k chunks) is woven
            # into the attention pair loops as PE filler under the
            # ACT-exp-bound windows. =====
            with (
                tc.tile_pool(name="xin", bufs=1) as x_pool,
                tc.tile_pool(name="wqk", bufs=1) as wqk_pool,
                tc.tile_pool(name="wv", bufs=1) as wv_pool,
                tc.tile_pool(name="wproj", bufs=1) as wproj_pool,
                tc.tile_pool(name="spk", bufs=4) as spk_pool,
                tc.tile_pool(name="texp", bufs=4) as t_pool,
                tc.tile_pool(name="pt", bufs=4) as pt_pool,
                tc.tile_pool(name="outT", bufs=1) as outT_pool,
                tc.tile_pool(name="rsmisc", bufs=2) as rs_pool,
            ):
                x_sb = x_pool.tile([128, KC, N], BF16)
                wqk_sb = wqk_pool.tile([128, KC, 2 * C], BF16)
                wv_sb = wv_pool.tile([128, KC, C], BF16)
                # x alone on the SP queue; wqk then wv on the ACT queue --
                # the two streams land in parallel so the qk matmuls (which
                # need x + wqk) start ~4us earlier. wv is only needed by
                # the v tiles woven into pair 0, much later.
                for kc in range(KC):
                    nc.sync.dma_start(x_sb[:, kc, :], xT_v[:, kc, :])
                for kc in range(KC):
                    nc.scalar.dma_start(wqk_sb[:, kc, :], wqkvT_v[:, kc, 0 : 2 * C])
                for kc in range(KC):
                    nc.scalar.dma_start(wv_sb[:, kc, :], wqkvT_v[:, kc, 2 * C :])

                # warm-up matmuls run while the DMAs stream
                for _ in range(18):
                    nc.tensor.matmul(
                        junk_ps[:, 0:512], lhsT=junk_sb[:, 0:128],
                        rhs=junk_sb[:], start=True, stop=True,
                    )

                for kc in range(KC):
                    gate(x_sb[:, kc, :])

                def emit_v(nt):
                    # v/qk PSUM tiles come only from the sc pool: the acc
                    # pool stays junk-only until the pair loop so the gate
                    # target (junk_ps) is never recycled under a gate.
                    v_ps = ps_tile(sc_psum, "sc")
                    for n0, nsz in ((0, 512), (512, 256)):
                        for kc in range(KC):
                            nc.tensor.matmul(
                                v_ps[:, n0 : n0 + nsz],
                                lhsT=x_sb[:, kc, nt * 128 : (nt + 1) * 128],
                                rhs=wv_sb[:, kc, n0 : n0 + nsz],
                                start=(kc == 0),
                                stop=(kc == KC - 1),
                            )
                    nc.vector.tensor_scalar(
                        vr_sb[:, nt, :],
                        v_ps[:, :C],
                        0.0,
                        float(T_STEPS),
                        mybir.AluOpType.max,
                        mybir.AluOpType.mult,
                    )

                def emit_qk(m, with_gates=False):
                    # kc-outer so the first chunk paces with the arriving
                    # wqk DMA chunks (gates woven) instead of waiting for
                    # the last one.
                    qk_ps = ps_tile(sc_psum, "sc")
                    for kc in range(KC):
                        if with_gates:
                            gate(wqk_sb[:, kc, :])
                        for h in range(NH):
                            sl = slice(h * 512, (h + 1) * 512)
                            nc.tensor.matmul(
                                qk_ps[:, sl],
                                lhsT=wqk_sb[:, kc, m * 128 : (m + 1) * 128],
                                rhs=x_sb[:, kc, sl],
                                start=(kc == 0),
                                stop=(kc == KC - 1),
                            )
                    nc.vector.tensor_scalar(
                        rqk_sb[:, m, :], qk_ps[:], 0.0, None, mybir.AluOpType.max
                    )

                # q,k for pair 0 only; the other 10 chunks are woven into
                # the pair loops below
                emit_qk(0, with_gates=True)
                emit_qk(NPAIR)
                # wv gates AFTER the upfront qk chunks: wv lands last on the
                # ACT queue and is first consumed by the v tiles in pair 0
                for kc in range(KC):
                    gate(wv_sb[:, kc, :])

                # ===== phase 2: attention, one head pair at a time =====
                outT_sb = outT_pool.tile([128, NPAIR, N], BF16)
                wp_sb = wproj_pool.tile([128, KC, C], BF16)

                gate(wfc1_sb[:])
                gate(wfc2_sb[:])
                for kc in range(KC):
                    nc.sync.dma_start(wp_sb[:, kc, :], wprojT_v[:, kc, :])
                    gate(wp_sb[:, kc, :])

                def emit_drain(p, rs_ps, pv_ps):
                    # rowsums were accumulated with 64 duplicated rows per
                    # head (A rows 0:64, B rows 64:128), so a plain [128,N]
                    # reciprocal straight from PSUM IS the per-head
                    # normalizer broadcast -- no DMA bounce needed.
                    rec_sb = rs_pool.tile([128, N], F32, tag="rec")
                    nc.vector.reciprocal_approx_fast(rec_sb[:], rs_ps[:])
                    nc.vector.tensor_tensor(
                        outT_sb[:, p, :], pv_ps[:], rec_sb[:],
                        mybir.AluOpType.mult,
                    )

                fc_cache = {}

                def emit_fc(p):
                    # fc1/fc2 as one 128x128 block-diagonal matmul per half;
                    # bias-adds on ACT so the boundary-critical DVE queue
                    # stays down to reciprocal+normalize
                    rq = rqk_sb[:, p, :]
                    rk = rqk_sb[:, NPAIR + p, :]
                    qs_ps = ps_tile(sc_psum, "sc")
                    ks_ps = ps_tile(sc_psum, "sc")
                    for ps_t, w_sb, r in ((qs_ps, wfc1_sb, rq), (ks_ps, wfc2_sb, rk)):
                        for h in range(NH):
                            sl = slice(h * 512, (h + 1) * 512)
                            nc.tensor.matmul(
                                ps_t[:, sl], lhsT=w_sb[:], rhs=r[:, sl],
                                start=True, stop=True,
                            )
                    qs_sb = spk_pool.tile([128, N], BF16, tag="spk")
                    ks_sb = spk_pool.tile([128, N], BF16, tag="spk")
                    nc.scalar.activation(
                        qs_sb[:], qs_ps[:], mybir.ActivationFunctionType.Identity,
                        bias=b1_sb[:, 0:1],
                    )
                    nc.scalar.activation(
                        ks_sb[:], ks_ps[:], mybir.ActivationFunctionType.Identity,
                        bias=b2_sb[:, 0:1],
                    )
                    fc_cache[p] = (qs_sb, ks_sb)

                emit_fc(0)
                drain_args = None
                for p in range(NPAIR):
                    hA, hB = 2 * p, 2 * p + 1

                    # previous pair's drain first: its recip/normalize land
                    # ahead of this pair's s-tile memsets in the DVE queue
                    if drain_args is not None:
                        emit_drain(*drain_args)

                    qs_sb, ks_sb = fc_cache.pop(p)

                    # rowsum + PV accumulators: no memset first-toucher; the
                    # first accumulating matmul's wait on the DVE max-op also
                    # covers the previous pair's recip/normalize reads
                    rs_ps = acc_psum.tile([128, N], F32, tag="acc")
                    pv_ps = acc_psum.tile([128, N], F32, tag="acc")

                    pt_A = pt_pool.tile([128, NT, N], BF16, tag="pt")
                    pt_B = pt_pool.tile([128, NT, N], BF16, tag="pt")

                    def emit_rspv(j):
                        st, sp = (j == 0), (j == NT - 1)
                        for h in range(NH):
                            sl = slice(h * 512, (h + 1) * 512)
                            # rowsums: A rows 0:64 (dup), B rows 64:128
                            nc.tensor.matmul(
                                rs_ps[0:64, sl], lhsT=ones_sb[:],
                                rhs=pt_A[:, j, sl], start=st, stop=sp,
                            )
                            nc.tensor.matmul(
                                rs_ps[64:128, sl], lhsT=ones_sb[:],
                                rhs=pt_B[:, j, sl], start=st, stop=sp,
                            )
                            # PV: A rows 0:64, B rows 64:128 (col-tiled)
                            nc.tensor.matmul(
                                pv_ps[0:64, sl],
                                lhsT=vr_sb[:, j, hA * HD : (hA + 1) * HD],
                                rhs=pt_A[:, j, sl], start=st, stop=sp,
                            )
                            nc.tensor.matmul(
                                pv_ps[64:128, sl],
                                lhsT=vr_sb[:, j, hB * HD : (hB + 1) * HD],
                                rhs=pt_B[:, j, sl], start=st, stop=sp,
                            )

                    # -- scores S^T[j, i] + exp + max(.,1), with rowsum/PV
                    # for tile jt-1 issued behind the scores for tile jt
                    for jt in range(NT):
                        jsl = slice(jt * 128, (jt + 1) * 128)
                        s_A = ps_tile(sc_psum, "sc")
                        s_B = ps_tile(sc_psum, "sc")
                        # issue order A-h0, B-h0, A-h1, B-h1: row-tiled A/B
                        # matmuls start back-to-back and run concurrently
                        for h in range(NH):
                            sl = slice(h * 512, (h + 1) * 512)
                            for base, s_ps2 in ((0, s_A), (64, s_B)):
                                nc.tensor.matmul(
                                    s_ps2[:, sl],
                                    lhsT=ks_sb[base : base + 64, jsl],
                                    rhs=qs_sb[base : base + 64, sl],
                                    start=True, stop=True,
                                )
                        t_a = t_pool.tile([128, N], BF16, tag="texp")
                        t_b = t_pool.tile([128, N], BF16, tag="texp")
                        nc.scalar.activation(
                            t_a[:], s_A[:], mybir.ActivationFunctionType.Exp
                        )
                        nc.vector.tensor_scalar(
                            pt_A[:, jt, :], t_a[:], 1.0, None, mybir.AluOpType.max
                        )
                        nc.scalar.activation(
                            t_b[:], s_B[:], mybir.ActivationFunctionType.Exp
                        )
                        nc.vector.tensor_scalar(
                            pt_B[:, jt, :], t_b[:], 1.0, None, mybir.AluOpType.max
                        )
                        # woven production work: PE filler under the
                        # ACT-exp-bound window of this jt slot
                        if p == 0:
                            emit_v(jt)
                        if p < NPAIR - 1:
                            if jt == 2:
                                emit_qk(p + 1)
                            elif jt == 5:
                                emit_qk(NPAIR + p + 1)
                            elif jt == 6:
                                emit_fc(p + 1)
                        if jt > 0:
                            emit_rspv(jt - 1)
                    emit_rspv(NT - 1)
                    drain_args = (p, rs_ps, pv_ps)
                emit_drain(*drain_args)

                # ================= phase 3: output projection =================
                with (
                    tc.tile_pool(name="yt", bufs=2) as y_pool,
                ):
                    for et in range(KC):
                        y_ps = ps_tile(sc_psum, "sc")
                        y_sb = y_pool.tile([128, N], F32, tag="yt")
                        # per-half bias + DMA (alternating queues): each
                        # half ships while the other half's matmuls run,
                        # halving the exposed tail of the last chunk
                        for h in range(NH):
                            sl = slice(h * 512, (h + 1) * 512)
                            for kc in range(KC):
                                nc.tensor.matmul(
                                    y_ps[:, sl],
                                    lhsT=wp_sb[:, kc, et * 128 : (et + 1) * 128],
                                    rhs=outT_sb[:, kc, sl],
                                    start=(kc == 0),
                                    stop=(kc == KC - 1),
                                )
                            nc.scalar.activation(
                                y_sb[:, sl], y_ps[:, sl],
                                mybir.ActivationFunctionType.Identity,
                                bias=bproj_sb[:, et : et + 1],
                            )
                            if h == 0:
                                nc.sync.dma_start(yT_v[:, et, sl], y_sb[:, sl])
                            else:
                                nc.scalar.dma_start(yT_v[:, et, sl], y_sb[:, sl])

    nc.compile()
    return nc


_NC_CACHE = {}


def _get_nc():
    if "nc" not in _NC_CACHE:
        _NC_CACHE["nc"] = build_nc()
    return _NC_CACHE["nc"]


def _make_in_maps(x, Wqkv, Wfc1, bfc1, Wfc2, bfc2, Wproj, bproj):
    bf = ml_dtypes.bfloat16
    f8 = ml_dtypes.float8_e4m3fn
    s2 = 2.0 * SCALE  # fold the *SCALE and the *N_HALF accumulation into Q path
    wqkvT = np.ascontiguousarray(Wqkv.T).astype(bf)
    wfc1p = np.zeros((128, 128), np.float32)
    wfc1p[0:64, 0:64] = Wfc1.T * s2
    wfc1p[64:128, 64:128] = Wfc1.T * s2
    wfc1p = wfc1p.astype(bf)
    wfc2p = np.zeros((128, 128), np.float32)
    wfc2p[0:64, 0:64] = Wfc2.T
    wfc2p[64:128, 64:128] = Wfc2.T
    wfc2p = wfc2p.astype(bf)
    b1p = np.concatenate([bfc1 * s2, bfc1 * s2]).astype(np.float32)[:, None]
    b2p = np.concatenate([bfc2, bfc2]).astype(np.float32)[:, None]
    wprojT = np.ascontiguousarray(Wproj.T).astype(bf)
    bprojp = np.ascontiguousarray(bproj.astype(np.float32).reshape(KC, 128).T)
    shared = dict(
        wqkvT=wqkvT, wfc1p=np.ascontiguousarray(wfc1p),
        wfc2p=np.ascontiguousarray(wfc2p), b1p=b1p, b2p=b2p,
        wprojT=wprojT, bprojp=bprojp,
    )
    maps = []
    for b in range(B):
        m = dict(shared)
        m["xT"] = np.ascontiguousarray(x[b].T).astype(bf)
        maps.append(m)
    return maps


def kernel(**inputs) -> np.ndarray:
    x = np.asarray(inputs["x"], dtype=np.float32)
    nc = _get_nc()
    in_maps = _make_in_maps(
        x,
        np.asarray(inputs["Wqkv"], np.float32),
        np.asarray(inputs["Wfc1"], np.float32),
        np.asarray(inputs["bfc1"], np.float32),
        np.asarray(inputs["Wfc2"], np.float32),
        np.asarray(inputs["bfc2"], np.float32),
        np.asarray(inputs["Wproj"], np.float32),
        np.asarray(inputs["bproj"], np.float32),
    )
    res = run_bass_kernel_spmd(nc, in_maps, core_ids=list(range(B)))
    out = np.empty((B, N, C), dtype=np.float32)
    for b in range(B):
        out[b] = res.results[b]["yT"].T
    return out
